# revision 2
# baseline (speedup 1.0000x reference)
"""TRN2 Bass kernel for nn_CausalAttention2Infusion (B=8, N=2048, D=DK=DV=1024).

att_b = softmax(causal(Q_b K_b^T / sqrt(DK))) V_b,  Q_b = x_b Wq^T, etc.
Data-parallel over batch: one batch element per NeuronCore, no collectives.

fp8e4 + DoubleRow matmuls everywhere, with a bf16 QK^T "island" protecting the
first 128 rows (their softmax has too few terms to average out fp8 error).

Score path uses associativity: S = x (Wq'^T Wk) x^T -> M (no x, hides the DMA),
Z = M^T x^T, V = x Wv^T; x^T, Z, V stay SBUF-resident in fp8.

Schedule: M -> Z -> ALL S^T strips (exp on ACT overlaps the next phases) ->
V8 -> island QK -> Vb -> PV groups (pure matmul + drain).

Scaling chain (fp8e4 normal range ~[1.6e-2, 224]):
  wq8 = Wq*norm*1024, wk8 = Wk*64 -> M8psum = M*65536; m8 = M8psum/32 = M*2048
  z8 = m8^T x8 = z*2048 (std ~21); S8 = x8 z8 = S*2048; e = exp(S8/2048)
  wv8 = Wv*32 -> v8 = v*32 (std ~18); ones column = 32 so att = num/den exact.

Hardware lessons baked in: every dma_start costs ~625ns of serial HWDGE ring
(few, large DMAs); DoubleRow moving operands need even subtile byte offsets
(v rows padded to 1026); DR and non-DR matmuls must not mix in one PSUM accum
group (odd PV groups ride a zero-padded DR pair); GPSIMD cannot touch PSUM.
"""
from contextlib import ExitStack

import numpy as np
import ml_dtypes

import concourse.mybir as mybir
import concourse.tile as tile
from concourse import bacc
from concourse.bass_utils import run_bass_kernel_spmd

F32 = mybir.dt.float32
BF16 = mybir.dt.bfloat16
FP8 = mybir.dt.float8e4
ALU = mybir.AluOpType
ACTF = mybir.ActivationFunctionType
DR = mybir.MatmulPerfMode.DoubleRow

P = 128
MASK_VAL = -1e30

B, N, D, DK, DV = 8, 2048, 1024, 1024, 1024
N_CORES = 8
SS = 1024              # super-strip width (i columns)

SC_WQ = 1024.0
SC_WK = 64.0
SC_M = 1.0 / 32.0      # m8 = M8psum * SC_M  (net m8 = M*2048)
SC_S = 2048.0
SC_V = 32.0


def _build_nc(N=N, D=D, DK=DK, DV=DV):
    nD, nK, nJ = D // P, DK // P, N // P
    nSS = N // SS
    SUB = SS // P          # 8 i-sub-blocks per super-strip
    CH = 512

    nc = bacc.Bacc("TRN2", target_bir_lowering=False, debug=False,
                   num_devices=N_CORES)

    def chunks_of(w):
        res, c0 = [], 0
        while c0 < w:
            res.append((c0, min(CH, w - c0)))
            c0 += CH
        return res

    xT8 = nc.dram_tensor("xT8", [D, N], FP8, kind="ExternalInput").ap()
    xTb = nc.dram_tensor("xTb", [D, P], BF16, kind="ExternalInput").ap()
    wq8 = nc.dram_tensor("wq8", [DK, D], FP8, kind="ExternalInput").ap()
    wk8 = nc.dram_tensor("wk8", [DK, D], FP8, kind="ExternalInput").ap()
    wvb = nc.dram_tensor("wvb", [D, DV], BF16, kind="ExternalInput").ap()
    wqT = nc.dram_tensor("wqT", [D, DK], BF16, kind="ExternalInput").ap()
    wkT = nc.dram_tensor("wkT", [D, DK], BF16, kind="ExternalInput").ap()
    out = nc.dram_tensor("out", [N, DV], F32, kind="ExternalOutput").ap()

    with tile.TileContext(nc) as tc, ExitStack() as ctx:
        resid = ctx.enter_context(tc.tile_pool(name="resid", bufs=1))
        wpool = ctx.enter_context(tc.tile_pool(name="wstream", bufs=3))
        epool = ctx.enter_context(tc.tile_pool(name="estrips", bufs=2))
        bigp = ctx.enter_context(tc.tile_pool(name="bigw", bufs=3))
        opool = ctx.enter_context(tc.tile_pool(name="attout", bufs=8))
        stat = ctx.enter_context(tc.tile_pool(name="stats", bufs=8))
        consts = ctx.enter_context(tc.tile_pool(name="consts", bufs=1))
        # PSUM budget: 8 banks = sch 2x[P,1024](4) + attA 2x[P,512](2)
        #              + attB 1x[P,512](1) + wrm 1x[P,512](1; warm + PV dens)
        psS = ctx.enter_context(tc.tile_pool(name="psS", bufs=2, space="PSUM"))
        psA = ctx.enter_context(tc.tile_pool(name="psA", bufs=2, space="PSUM"))
        psB = ctx.enter_context(tc.tile_pool(name="psB", bufs=1, space="PSUM"))
        psW = ctx.enter_context(tc.tile_pool(name="psW", bufs=1, space="PSUM"))

        xt_sb = resid.tile([P, nD, N], FP8)
        xtb_sb = resid.tile([P, nD, P], BF16)
        z_sb = resid.tile([P, nD, N], FP8)
        # row length padded to an even 1026 bytes: DoubleRow fetches the two
        # k-subtile streams in lockstep and faults on odd subtile byte offsets
        v_sb = resid.tile([P, nJ, DV + 2], FP8)
        vb_sb = resid.tile([P, DV + 1], BF16)
        qt_sb = resid.tile([P, nK, P], BF16)
        kt_sb = resid.tile([P, nK, P], BF16)
        wv8_sb = resid.tile([P, nD, DV], FP8)

        # warm-up matmuls on a zero tile during the initial input DMA keep the
        # PE p-state ramped so real matmuls start at full clock
        warm = consts.tile([P, P], BF16)
        nc.vector.memset(warm[:], 0.0)
        ps_w = psW.tile([P, CH], F32, tag="wrm", name="ps_w")
        for _ in range(46):
            nc.tensor.matmul(ps_w[:, 0:P], warm[:], warm[:],
                             start=True, stop=True)

        # triangular mask for the diagonal 128x128 window of each j-block:
        # tri[jj, ii] = (jj > ii) ? MASK_VAL : 0  — only that window is ever
        # partially masked; columns left of it are skipped, right are visible
        tri = consts.tile([P, P], F32)
        nc.gpsimd.memset(tri[:], 0.0)
        nc.gpsimd.affine_select(
            out=tri[:], in_=tri[:],
            compare_op=ALU.is_ge, fill=MASK_VAL, base=0,
            pattern=[[1, P]], channel_multiplier=-1,
        )
        eb_sb = consts.tile([P, P], BF16)   # exact-island e block (j,i < 128)
        nc.gpsimd.memset(v_sb[:, :, DV:DV + 1], SC_V)
        nc.gpsimd.memset(vb_sb[:, DV:DV + 1], 1.0)
        # zero-padded stationary pair for odd PV groups (uniform DR groups)
        epad = consts.tile([P, 2, P], FP8)
        nc.gpsimd.memset(epad[:], 0.0)

        # ---- input DMAs, priority order, all on the SP ring ----
        # few, large DMAs: every dma_start costs ~625ns of serial HWDGE ring
        wq_sb = wpool.tile([P, nK, D], FP8, tag="w")
        wk_sb = wpool.tile([P, nK, D], FP8, tag="w")
        wqN_t = wq8.rearrange("(t p) d -> p t d", p=P)
        wkN_t = wk8.rearrange("(t p) d -> p t d", p=P)
        h = nK // 2
        nc.sync.dma_start(wq_sb[:, 0:h], wqN_t[:, 0:h])
        nc.sync.dma_start(wk_sb[:, 0:h], wkN_t[:, 0:h])
        nc.sync.dma_start(wq_sb[:, h:nK], wqN_t[:, h:nK])
        nc.sync.dma_start(wk_sb[:, h:nK], wkN_t[:, h:nK])
        xT_t = xT8.rearrange("(t p) n -> p t n", p=P)
        for (c0, cw) in chunks_of(N):   # chunk-major: d-tiles of chunk 0 first
            nc.sync.dma_start(xt_sb[:, :, c0:c0 + cw], xT_t[:, :, c0:c0 + cw])
        wvb_sb = bigp.tile([P, nD, DV], BF16, tag="bw")
        nc.sync.dma_start(wvb_sb[:], wvb.rearrange("(t p) v -> p t v", p=P))
        nc.sync.dma_start(xtb_sb[:], xTb.rearrange("(t p) j -> p t j", p=P))
        wqT_sb = bigp.tile([P, nD, DK], BF16, tag="bw")
        nc.sync.dma_start(wqT_sb[:], wqT.rearrange("(t p) k -> p t k", p=P))
        wkT_sb = bigp.tile([P, nD, DK], BF16, tag="bw")
        nc.sync.dma_start(wkT_sb[:], wkT.rearrange("(t p) k -> p t k", p=P))

        # phase-1 512-wide PSUM tiles cycle over psS/psA/psB slots (psA/psB
        # are idle until the PV phase) for 5-deep pipelining
        _ps_cycle = [0]
        _ps_pools = [(psS, "sch"), (psA, "attA"), (psB, "attB")]

        def ps512(name):
            pool, tag = _ps_pools[_ps_cycle[0] % len(_ps_pools)]
            _ps_cycle[0] += 1
            return pool.tile([P, CH], F32, tag=tag, name=name)

        # ---- phase 1a: M8[d',d] = sum_k wq8 wk8 (DoubleRow k-pairs) ----
        # d1t==0 is paced by the wq/wk DMA: filler matmuls bridge the stream
        # so the PE p-state never drops
        m_sb = wpool.tile([P, nK, D], FP8, tag="w")
        d_chunks = chunks_of(D)
        for d1t in range(nD):
            pss = [ps512(f"mps{i}") for i in range(len(d_chunks))]
            for kt in range(0, nK, 2):
                for ic, (c0, cw) in enumerate(d_chunks):
                    nc.tensor.matmul(
                        pss[ic][:, :cw],
                        wq_sb[:, kt:kt + 2, d1t * P:(d1t + 1) * P],
                        wk_sb[:, kt:kt + 2, c0:c0 + cw],
                        start=(kt == 0), stop=(kt == nK - 2),
                        perf_mode=DR,
                    )
            for ic, (c0, cw) in enumerate(d_chunks):
                if (d1t + ic) % 2 == 0:
                    nc.vector.tensor_scalar_mul(
                        m_sb[:, d1t, c0:c0 + cw], pss[ic][:, :cw], SC_M)
                else:
                    nc.scalar.activation(
                        m_sb[:, d1t, c0:c0 + cw], pss[ic][:, :cw],
                        ACTF.Copy, scale=SC_M)

        # wv8 = wvb cast to fp8*SC_V on the (otherwise idle) Pool engine
        for d in range(nD):
            nc.gpsimd.tensor_scalar_mul(wv8_sb[:, d], wvb_sb[:, d], SC_V)

        # bridge the M->Z transition (copy drain) without dropping p-state
        for _ in range(10):
            nc.tensor.matmul(ps_w[:, 0:P], warm[:], warm[:],
                             start=True, stop=True)

        # ---- phase 1b: z8[d,i] = sum_d' m8 x8 (chunk-major, DR pairs) ----
        for ic, (c0, cw) in enumerate(chunks_of(N)):
            for dt in range(nD):
                ps = ps512("zps")
                for dp in range(0, nD, 2):
                    nc.tensor.matmul(
                        ps[:, :cw],
                        m_sb[:, dp:dp + 2, dt * P:(dt + 1) * P],
                        xt_sb[:, dp:dp + 2, c0:c0 + cw],
                        start=(dp == 0), stop=(dp == nD - 2),
                        perf_mode=DR,
                    )
                if (ic + dt) % 2 == 0:
                    nc.vector.tensor_copy(z_sb[:, dt, c0:c0 + cw], ps[:, :cw])
                else:
                    nc.scalar.copy(z_sb[:, dt, c0:c0 + cw], ps[:, :cw])

        # ---- S^T strips interleaved with V8/island/Vb matmul units: the S
        # chain drains through the ACT exp (its rate limit), so independent
        # PE work units slot into the gaps instead of head-of-line blocking.
        # S blocks use the two wide [P,1024] psS slots; the other units use
        # psA/psB, so the two streams never contend for PSUM. ----
        _ps_pools[:] = [(psA, "attA"), (psB, "attB")]
        v_chunks = chunks_of(DV)
        e_strips = [epool.tile([P, nJ, SS], FP8, tag="e", name=f"e{I}")
                    for I in range(nSS)]

        def s_unit(I, jt):
            e_sb = e_strips[I]
            c = jt - SUB * I
            i0 = c * P if c > 0 else 0
            ps = psS.tile([P, SS], F32, tag="sch", name="sps")
            segs = []
            s0 = i0
            while s0 < SS:
                sw = min(CH - (s0 % CH), SS - s0)
                segs.append((s0, sw))
                s0 += sw
            for k in range(0, nD, 2):
                for (s0, sw) in segs:
                    nc.tensor.matmul(
                        ps[:, s0:s0 + sw],
                        xt_sb[:, k:k + 2, jt * P:(jt + 1) * P],
                        z_sb[:, k:k + 2, I * SS + s0:I * SS + s0 + sw],
                        start=(k == 0), stop=(k == nD - 2),
                        perf_mode=DR,
                    )
            if c >= 0:
                nc.vector.tensor_add(ps[:, i0:i0 + P], ps[:, i0:i0 + P],
                                     tri[:])
            nc.scalar.activation(e_sb[:, jt, i0:SS], ps[:, i0:SS],
                                 ACTF.Exp, scale=1.0 / SC_S)

        def v8_unit(jt):
            pss = [ps512(f"vps{i}") for i in range(len(v_chunks))]
            for d in range(0, nD, 2):
                for vc, (c0, cw) in enumerate(v_chunks):
                    nc.tensor.matmul(
                        pss[vc][:, :cw],
                        xt_sb[:, d:d + 2, jt * P:(jt + 1) * P],
                        wv8_sb[:, d:d + 2, c0:c0 + cw],
                        start=(d == 0), stop=(d == nD - 2),
                        perf_mode=DR,
                    )
            for vc, (c0, cw) in enumerate(v_chunks):
                if (jt + vc) % 2 == 0:
                    nc.vector.tensor_copy(v_sb[:, jt, c0:c0 + cw],
                                          pss[vc][:, :cw])
                else:
                    nc.scalar.copy(v_sb[:, jt, c0:c0 + cw], pss[vc][:, :cw])

        def proj_unit(w_sb, dst, kt):
            ps = ps512("pjps")
            for dp in range(nD):
                nc.tensor.matmul(ps[:, :P],
                                 w_sb[:, dp, kt * P:(kt + 1) * P],
                                 xtb_sb[:, dp, :],
                                 start=(dp == 0), stop=(dp == nD - 1))
            if kt % 2 == 0:
                nc.vector.tensor_copy(dst[:, kt], ps[:, :P])
            else:
                nc.scalar.copy(dst[:, kt], ps[:, :P])

        def isl_unit():
            ps_isl = ps512("islps")
            for kt in range(nK):
                nc.tensor.matmul(ps_isl[:, :P], kt_sb[:, kt], qt_sb[:, kt],
                                 start=(kt == 0), stop=(kt == nK - 1))
            nc.vector.tensor_add(ps_isl[:, :P], ps_isl[:, :P], tri[:])
            nc.scalar.activation(eb_sb[:], ps_isl[:, :P], ACTF.Exp)

        def vb_unit():
            pss = [ps512(f"vbps{i}") for i in range(len(v_chunks))]
            for d in range(nD):
                for vc, (c0, cw) in enumerate(v_chunks):
                    nc.tensor.matmul(
                        pss[vc][:, :cw],
                        xtb_sb[:, d, :],
                        wvb_sb[:, d, c0:c0 + cw],
                        start=(d == 0), stop=(d == nD - 1),
                    )
            for vc, (c0, cw) in enumerate(v_chunks):
                if vc % 2 == 0:
                    nc.vector.tensor_copy(vb_sb[:, c0:c0 + cw], pss[vc][:, :cw])
                else:
                    nc.scalar.copy(vb_sb[:, c0:c0 + cw], pss[vc][:, :cw])

        s_units = [(I, jt) for I in range(nSS) for jt in range(SUB * I + SUB)]
        w_units = ([("v8", jt) for jt in range(nJ)]
                   + [("qt", kt) for kt in range(nK)]
                   + [("kt", kt) for kt in range(nK)]
                   + [("isl", 0), ("vb", 0)])
        nW, nS = len(w_units), len(s_units)
        wi = 0
        for si, (I, jt) in enumerate(s_units):
            s_unit(I, jt)
            target = nW * (si + 1) // nS
            while wi < target:
                kind, a = w_units[wi]
                if kind == "v8":
                    v8_unit(a)
                elif kind == "qt":
                    proj_unit(wqT_sb, qt_sb, a)
                elif kind == "kt":
                    proj_unit(wkT_sb, kt_sb, a)
                elif kind == "isl":
                    isl_unit()
                else:
                    vb_unit()
                wi += 1

        # ---- PV groups: pass A (v hi cols + den) then pass B (v lo cols);
        # rcp + A-scale hide under B's accumulation; A double-buffered ----
        pv_order = []
        for c in range(SUB):
            for I in range(nSS):
                pv_order.append((I, c))
        for (I, c) in pv_order:
                e_sb = e_strips[I]
                npv = SUB * I + c + 1
                psa = psA.tile([P, CH], F32, tag="attA")
                dpool = psW if (SUB * I + c) % 2 == 0 else psB
                dtag = "wrm" if (SUB * I + c) % 2 == 0 else "attB"
                psd = dpool.tile([P, 4], F32, tag=dtag, name="psd")
                psb = psS.tile([P, CH], F32, tag="sch", name="psb")
                row0 = I * SS + c * P
                rcp = stat.tile([P, 1], F32, tag="rcp")
                o_hi = opool.tile([P, CH], F32, tag="o")
                o_lo = opool.tile([P, CH], F32, tag="o")
                if I == 0 and c == 0:
                    # exact bf16 island: rows 0..127 attend only j-block 0
                    nc.tensor.matmul(psd[:, 0:1], eb_sb[:],
                                     vb_sb[:, DV:DV + 1],
                                     start=True, stop=True)
                    nc.vector.reciprocal(rcp[:], psd[:, 0:1])
                    nc.tensor.matmul(psa[:, 0:CH], eb_sb[:], vb_sb[:, CH:DV],
                                     start=True, stop=True)
                    nc.tensor.matmul(psb[:, 0:CH], eb_sb[:], vb_sb[:, 0:CH],
                                     start=True, stop=True)
                else:
                    odd = npv % 2
                    if odd:
                        # j-block 0 rides the zero-padded pair
                        nc.gpsimd.tensor_copy(epad[:, 0],
                                              e_sb[:, 0, c * P:(c + 1) * P])
                        stats = [(epad[:, :, :], 0)]
                        stats += [(e_sb[:, jt:jt + 2, c * P:(c + 1) * P], jt)
                                  for jt in range(1, npv - 1, 2)]
                    else:
                        stats = [(e_sb[:, jt:jt + 2, c * P:(c + 1) * P], jt)
                                 for jt in range(0, npv - 1, 2)]
                    # denominator pass first: the reciprocal runs on DVE while
                    # the PE streams the wide A/B passes
                    for ji, (st, jt) in enumerate(stats):
                        first, last = (ji == 0), (ji == len(stats) - 1)
                        nc.tensor.matmul(psd[:, 0:1], st,
                                         v_sb[:, jt:jt + 2, DV:DV + 1],
                                         start=first, stop=last,
                                         perf_mode=DR)
                    nc.vector.reciprocal(rcp[:], psd[:, 0:1])
                    for ji, (st, jt) in enumerate(stats):
                        first, last = (ji == 0), (ji == len(stats) - 1)
                        nc.tensor.matmul(psa[:, 0:CH], st,
                                         v_sb[:, jt:jt + 2, CH:DV],
                                         start=first, stop=last,
                                         perf_mode=DR)
                    for ji, (st, jt) in enumerate(stats):
                        first, last = (ji == 0), (ji == len(stats) - 1)
                        nc.tensor.matmul(psb[:, 0:CH], st,
                                         v_sb[:, jt:jt + 2, 0:CH],
                                         start=first, stop=last,
                                         perf_mode=DR)
                if (SUB * I + c) % 2 == 0:
                    nc.vector.tensor_scalar_mul(o_hi[:], psa[:, 0:CH], rcp[:])
                    nc.scalar.activation(o_lo[:], psb[:, 0:CH],
                                         ACTF.Copy, scale=rcp[:])
                else:
                    nc.scalar.activation(o_hi[:], psa[:, 0:CH],
                                         ACTF.Copy, scale=rcp[:])
                    nc.vector.tensor_scalar_mul(o_lo[:], psb[:, 0:CH], rcp[:])
                nc.sync.dma_start(out[row0:row0 + P, CH:DV], o_hi[:])
                nc.sync.dma_start(out[row0:row0 + P, 0:CH], o_lo[:])

    nc.compile()
    return nc


_NC_CACHE = {}


def _get_nc():
    if "nc" not in _NC_CACHE:
        _NC_CACHE["nc"] = _build_nc()
    return _NC_CACHE["nc"]


def kernel(x, Wq, Wk, Wv):
    x = np.asarray(x, dtype=np.float32)
    Wq = np.asarray(Wq, dtype=np.float32)
    Wk = np.asarray(Wk, dtype=np.float32)
    Wv = np.asarray(Wv, dtype=np.float32)
    assert x.shape == (B, N, D), x.shape

    nc = _get_nc()
    bf = ml_dtypes.bfloat16
    f8 = ml_dtypes.float8_e4m3
    norm = np.float32(1.0) / np.sqrt(np.float32(DK))
    wq8_h = np.ascontiguousarray(Wq * (norm * SC_WQ)).astype(f8)
    wk8_h = np.ascontiguousarray(Wk * SC_WK).astype(f8)
    wvb_h = np.ascontiguousarray(Wv.T).astype(bf)
    wqT_h = np.ascontiguousarray((Wq * norm).T).astype(bf)
    wkT_h = np.ascontiguousarray(Wk.T).astype(bf)
    in_maps = []
    for b in range(B):
        xt = np.ascontiguousarray(x[b].T)
        in_maps.append({
            "xT8": xt.astype(f8),
            "xTb": np.ascontiguousarray(xt[:, :P]).astype(bf),
            "wq8": wq8_h, "wk8": wk8_h, "wvb": wvb_h,
            "wqT": wqT_h, "wkT": wkT_h,
        })
    res = run_bass_kernel_spmd(nc, in_maps, list(range(N_CORES)))
    return np.stack([res.results[b]["out"] for b in range(B)], axis=0)


# revision 3
# speedup vs baseline: 1.0020x; 1.0020x over previous
"""TRN2 Bass kernel for nn_CausalAttention2Infusion (B=8, N=2048, D=DK=DV=1024).

att_b = softmax(causal(Q_b K_b^T / sqrt(DK))) V_b,  Q_b = x_b Wq^T, etc.
Data-parallel over batch: one batch element per NeuronCore, no collectives.

fp8e4 + DoubleRow matmuls everywhere, with a bf16 QK^T "island" protecting the
first 128 rows (their softmax has too few terms to average out fp8 error).

Score path uses associativity: S = x (Wq'^T Wk) x^T -> M (no x, hides the DMA),
Z = M^T x^T, V = x Wv^T; x^T, Z, V stay SBUF-resident in fp8.

Schedule: M -> Z -> ALL S^T strips (exp on ACT overlaps the next phases) ->
V8 -> island QK -> Vb -> PV groups (pure matmul + drain).

Scaling chain (fp8e4 normal range ~[1.6e-2, 224]):
  wq8 = Wq*norm*1024, wk8 = Wk*64 -> M8psum = M*65536; m8 = M8psum/32 = M*2048
  z8 = m8^T x8 = z*2048 (std ~21); S8 = x8 z8 = S*2048; e = exp(S8/2048)
  wv8 = Wv*32 -> v8 = v*32 (std ~18); ones column = 32 so att = num/den exact.

Hardware lessons baked in: every dma_start costs ~625ns of serial HWDGE ring
(few, large DMAs); DoubleRow moving operands need even subtile byte offsets
(v rows padded to 1026); DR and non-DR matmuls must not mix in one PSUM accum
group (odd PV groups ride a zero-padded DR pair); GPSIMD cannot touch PSUM.
"""
from contextlib import ExitStack

import numpy as np
import ml_dtypes

import concourse.mybir as mybir
import concourse.tile as tile
from concourse import bacc
from concourse.bass_utils import run_bass_kernel_spmd

F32 = mybir.dt.float32
BF16 = mybir.dt.bfloat16
FP8 = mybir.dt.float8e4
ALU = mybir.AluOpType
ACTF = mybir.ActivationFunctionType
DR = mybir.MatmulPerfMode.DoubleRow

P = 128
MASK_VAL = -1e30

B, N, D, DK, DV = 8, 2048, 1024, 1024, 1024
N_CORES = 8
SS = 1024              # super-strip width (i columns)

SC_WQ = 1024.0
SC_WK = 64.0
SC_M = 1.0 / 32.0      # m8 = M8psum * SC_M  (net m8 = M*2048)
SC_S = 2048.0
SC_V = 32.0


def _build_nc(N=N, D=D, DK=DK, DV=DV):
    nD, nK, nJ = D // P, DK // P, N // P
    nSS = N // SS
    SUB = SS // P          # 8 i-sub-blocks per super-strip
    CH = 512

    nc = bacc.Bacc("TRN2", target_bir_lowering=False, debug=False,
                   num_devices=N_CORES)

    def chunks_of(w):
        res, c0 = [], 0
        while c0 < w:
            res.append((c0, min(CH, w - c0)))
            c0 += CH
        return res

    xT8 = nc.dram_tensor("xT8", [D, N], FP8, kind="ExternalInput").ap()
    xTb = nc.dram_tensor("xTb", [D, P], BF16, kind="ExternalInput").ap()
    wq8 = nc.dram_tensor("wq8", [DK, D], FP8, kind="ExternalInput").ap()
    wk8 = nc.dram_tensor("wk8", [DK, D], FP8, kind="ExternalInput").ap()
    wvb = nc.dram_tensor("wvb", [D, DV], BF16, kind="ExternalInput").ap()
    wqT = nc.dram_tensor("wqT", [D, DK], BF16, kind="ExternalInput").ap()
    wkT = nc.dram_tensor("wkT", [D, DK], BF16, kind="ExternalInput").ap()
    out = nc.dram_tensor("out", [N, DV], F32, kind="ExternalOutput").ap()

    with tile.TileContext(nc) as tc, ExitStack() as ctx:
        resid = ctx.enter_context(tc.tile_pool(name="resid", bufs=1))
        wpool = ctx.enter_context(tc.tile_pool(name="wstream", bufs=3))
        epool = ctx.enter_context(tc.tile_pool(name="estrips", bufs=2))
        bigp = ctx.enter_context(tc.tile_pool(name="bigw", bufs=3))
        opool = ctx.enter_context(tc.tile_pool(name="attout", bufs=8))
        stat = ctx.enter_context(tc.tile_pool(name="stats", bufs=8))
        consts = ctx.enter_context(tc.tile_pool(name="consts", bufs=1))
        # PSUM budget: 8 banks = sch 2x[P,1024](4) + attA 2x[P,512](2)
        #              + attB 1x[P,512](1) + wrm 1x[P,512](1; warm + PV dens)
        psS = ctx.enter_context(tc.tile_pool(name="psS", bufs=2, space="PSUM"))
        psA = ctx.enter_context(tc.tile_pool(name="psA", bufs=2, space="PSUM"))
        psB = ctx.enter_context(tc.tile_pool(name="psB", bufs=1, space="PSUM"))
        psW = ctx.enter_context(tc.tile_pool(name="psW", bufs=1, space="PSUM"))

        xt_sb = resid.tile([P, nD, N], FP8)
        xtb_sb = resid.tile([P, nD, P], BF16)
        z_sb = resid.tile([P, nD, N], FP8)
        # row length padded to an even 1026 bytes: DoubleRow fetches the two
        # k-subtile streams in lockstep and faults on odd subtile byte offsets
        v_sb = resid.tile([P, nJ, DV + 2], FP8)
        vb_sb = resid.tile([P, DV + 1], BF16)
        qt_sb = resid.tile([P, nK, P], BF16)
        kt_sb = resid.tile([P, nK, P], BF16)
        wv8_sb = resid.tile([P, nD, DV], FP8)

        # warm-up matmuls on a zero tile during the initial input DMA keep the
        # PE p-state ramped so real matmuls start at full clock
        warm = consts.tile([P, P], BF16)
        nc.vector.memset(warm[:], 0.0)
        ps_w = psW.tile([P, CH], F32, tag="wrm", name="ps_w")
        for _ in range(30):
            nc.tensor.matmul(ps_w[:, 0:P], warm[:], warm[:],
                             start=True, stop=True)

        # triangular mask for the diagonal 128x128 window of each j-block:
        # tri[jj, ii] = (jj > ii) ? MASK_VAL : 0  — only that window is ever
        # partially masked; columns left of it are skipped, right are visible
        tri = consts.tile([P, P], F32)
        nc.gpsimd.memset(tri[:], 0.0)
        nc.gpsimd.affine_select(
            out=tri[:], in_=tri[:],
            compare_op=ALU.is_ge, fill=MASK_VAL, base=0,
            pattern=[[1, P]], channel_multiplier=-1,
        )
        eb_sb = consts.tile([P, P], BF16)   # exact-island e block (j,i < 128)
        nc.gpsimd.memset(v_sb[:, :, DV:DV + 1], SC_V)
        nc.gpsimd.memset(vb_sb[:, DV:DV + 1], 1.0)
        # zero-padded stationary pair for odd PV groups (uniform DR groups)
        epad = consts.tile([P, 2, P], FP8)
        nc.gpsimd.memset(epad[:], 0.0)

        # ---- input DMAs, priority order, all on the SP ring ----
        # few, large DMAs: every dma_start costs ~625ns of serial HWDGE ring
        wq_sb = wpool.tile([P, nK, D], FP8, tag="w")
        wk_sb = wpool.tile([P, nK, D], FP8, tag="w")
        wqN_t = wq8.rearrange("(t p) d -> p t d", p=P)
        wkN_t = wk8.rearrange("(t p) d -> p t d", p=P)
        for q0 in range(0, nK, 2):   # quarter-granularity: M starts ~2us sooner
            nc.sync.dma_start(wq_sb[:, q0:q0 + 2], wqN_t[:, q0:q0 + 2])
            nc.sync.dma_start(wk_sb[:, q0:q0 + 2], wkN_t[:, q0:q0 + 2])
        xT_t = xT8.rearrange("(t p) n -> p t n", p=P)
        for (c0, cw) in chunks_of(N):   # chunk-major: d-tiles of chunk 0 first
            nc.sync.dma_start(xt_sb[:, :, c0:c0 + cw], xT_t[:, :, c0:c0 + cw])
        wvb_sb = bigp.tile([P, nD, DV], BF16, tag="bw")
        nc.sync.dma_start(wvb_sb[:], wvb.rearrange("(t p) v -> p t v", p=P))
        nc.sync.dma_start(xtb_sb[:], xTb.rearrange("(t p) j -> p t j", p=P))
        wqT_sb = bigp.tile([P, nD, DK], BF16, tag="bw")
        nc.sync.dma_start(wqT_sb[:], wqT.rearrange("(t p) k -> p t k", p=P))
        wkT_sb = bigp.tile([P, nD, DK], BF16, tag="bw")
        nc.sync.dma_start(wkT_sb[:], wkT.rearrange("(t p) k -> p t k", p=P))

        # phase-1 512-wide PSUM tiles cycle over psS/psA/psB slots (psA/psB
        # are idle until the PV phase) for 5-deep pipelining
        _ps_cycle = [0]
        _ps_pools = [(psS, "sch"), (psA, "attA"), (psB, "attB")]

        def ps512(name):
            pool, tag = _ps_pools[_ps_cycle[0] % len(_ps_pools)]
            _ps_cycle[0] += 1
            return pool.tile([P, CH], F32, tag=tag, name=name)

        # ---- phase 1a: M8[d',d] = sum_k wq8 wk8 (DoubleRow k-pairs) ----
        # d1t==0 is paced by the wq/wk DMA: filler matmuls bridge the stream
        # so the PE p-state never drops
        m_sb = wpool.tile([P, nK, D], FP8, tag="w")
        d_chunks = chunks_of(D)
        for d1t in range(nD):
            pss = [ps512(f"mps{i}") for i in range(len(d_chunks))]
            for kt in range(0, nK, 2):
                for ic, (c0, cw) in enumerate(d_chunks):
                    nc.tensor.matmul(
                        pss[ic][:, :cw],
                        wq_sb[:, kt:kt + 2, d1t * P:(d1t + 1) * P],
                        wk_sb[:, kt:kt + 2, c0:c0 + cw],
                        start=(kt == 0), stop=(kt == nK - 2),
                        perf_mode=DR,
                    )
            for ic, (c0, cw) in enumerate(d_chunks):
                if (d1t + ic) % 2 == 0:
                    nc.vector.tensor_scalar_mul(
                        m_sb[:, d1t, c0:c0 + cw], pss[ic][:, :cw], SC_M)
                else:
                    nc.scalar.activation(
                        m_sb[:, d1t, c0:c0 + cw], pss[ic][:, :cw],
                        ACTF.Copy, scale=SC_M)

        # wv8 = wvb cast to fp8*SC_V on the (otherwise idle) Pool engine
        for d in range(nD):
            nc.gpsimd.tensor_scalar_mul(wv8_sb[:, d], wvb_sb[:, d], SC_V)

        # bridge the M->Z transition (copy drain) without dropping p-state
        for _ in range(10):
            nc.tensor.matmul(ps_w[:, 0:P], warm[:], warm[:],
                             start=True, stop=True)

        # ---- phase 1b: z8[d,i] = sum_d' m8 x8 (chunk-major, DR pairs) ----
        for ic, (c0, cw) in enumerate(chunks_of(N)):
            for dt in range(nD):
                ps = ps512("zps")
                for dp in range(0, nD, 2):
                    nc.tensor.matmul(
                        ps[:, :cw],
                        m_sb[:, dp:dp + 2, dt * P:(dt + 1) * P],
                        xt_sb[:, dp:dp + 2, c0:c0 + cw],
                        start=(dp == 0), stop=(dp == nD - 2),
                        perf_mode=DR,
                    )
                if (ic + dt) % 2 == 0:
                    nc.vector.tensor_copy(z_sb[:, dt, c0:c0 + cw], ps[:, :cw])
                else:
                    nc.scalar.copy(z_sb[:, dt, c0:c0 + cw], ps[:, :cw])

        # ---- S^T strips interleaved with V8/island/Vb matmul units: the S
        # chain drains through the ACT exp (its rate limit), so independent
        # PE work units slot into the gaps instead of head-of-line blocking.
        # S blocks use the two wide [P,1024] psS slots; the other units use
        # psA/psB, so the two streams never contend for PSUM. ----
        _ps_pools[:] = [(psA, "attA"), (psB, "attB")]
        v_chunks = chunks_of(DV)
        e_strips = [epool.tile([P, nJ, SS], FP8, tag="e", name=f"e{I}")
                    for I in range(nSS)]

        def s_unit(I, jt):
            e_sb = e_strips[I]
            c = jt - SUB * I
            i0 = c * P if c > 0 else 0
            ps = psS.tile([P, SS], F32, tag="sch", name="sps")
            segs = []
            s0 = i0
            while s0 < SS:
                sw = min(CH - (s0 % CH), SS - s0)
                segs.append((s0, sw))
                s0 += sw
            for k in range(0, nD, 2):
                for (s0, sw) in segs:
                    nc.tensor.matmul(
                        ps[:, s0:s0 + sw],
                        xt_sb[:, k:k + 2, jt * P:(jt + 1) * P],
                        z_sb[:, k:k + 2, I * SS + s0:I * SS + s0 + sw],
                        start=(k == 0), stop=(k == nD - 2),
                        perf_mode=DR,
                    )
            if c >= 0:
                nc.vector.tensor_add(ps[:, i0:i0 + P], ps[:, i0:i0 + P],
                                     tri[:])
            nc.scalar.activation(e_sb[:, jt, i0:SS], ps[:, i0:SS],
                                 ACTF.Exp, scale=1.0 / SC_S)

        def v8_unit(jt):
            pss = [ps512(f"vps{i}") for i in range(len(v_chunks))]
            for d in range(0, nD, 2):
                for vc, (c0, cw) in enumerate(v_chunks):
                    nc.tensor.matmul(
                        pss[vc][:, :cw],
                        xt_sb[:, d:d + 2, jt * P:(jt + 1) * P],
                        wv8_sb[:, d:d + 2, c0:c0 + cw],
                        start=(d == 0), stop=(d == nD - 2),
                        perf_mode=DR,
                    )
            for vc, (c0, cw) in enumerate(v_chunks):
                if (jt + vc) % 2 == 0:
                    nc.vector.tensor_copy(v_sb[:, jt, c0:c0 + cw],
                                          pss[vc][:, :cw])
                else:
                    nc.scalar.copy(v_sb[:, jt, c0:c0 + cw], pss[vc][:, :cw])

        def proj_unit(w_sb, dst, kt):
            ps = ps512("pjps")
            for dp in range(nD):
                nc.tensor.matmul(ps[:, :P],
                                 w_sb[:, dp, kt * P:(kt + 1) * P],
                                 xtb_sb[:, dp, :],
                                 start=(dp == 0), stop=(dp == nD - 1))
            if kt % 2 == 0:
                nc.vector.tensor_copy(dst[:, kt], ps[:, :P])
            else:
                nc.scalar.copy(dst[:, kt], ps[:, :P])

        def isl_unit():
            ps_isl = ps512("islps")
            for kt in range(nK):
                nc.tensor.matmul(ps_isl[:, :P], kt_sb[:, kt], qt_sb[:, kt],
                                 start=(kt == 0), stop=(kt == nK - 1))
            nc.vector.tensor_add(ps_isl[:, :P], ps_isl[:, :P], tri[:])
            nc.scalar.activation(eb_sb[:], ps_isl[:, :P], ACTF.Exp)

        def vb_unit():
            pss = [ps512(f"vbps{i}") for i in range(len(v_chunks))]
            for d in range(nD):
                for vc, (c0, cw) in enumerate(v_chunks):
                    nc.tensor.matmul(
                        pss[vc][:, :cw],
                        xtb_sb[:, d, :],
                        wvb_sb[:, d, c0:c0 + cw],
                        start=(d == 0), stop=(d == nD - 1),
                    )
            for vc, (c0, cw) in enumerate(v_chunks):
                if vc % 2 == 0:
                    nc.vector.tensor_copy(vb_sb[:, c0:c0 + cw], pss[vc][:, :cw])
                else:
                    nc.scalar.copy(vb_sb[:, c0:c0 + cw], pss[vc][:, :cw])

        s_units = [(I, jt) for I in range(nSS) for jt in range(SUB * I + SUB)]
        w_units = ([("v8", jt) for jt in range(nJ)]
                   + [("qt", kt) for kt in range(nK)]
                   + [("kt", kt) for kt in range(nK)]
                   + [("isl", 0), ("vb", 0)])
        nW, nS = len(w_units), len(s_units)
        wi = 0
        for si, (I, jt) in enumerate(s_units):
            s_unit(I, jt)
            target = nW * (si + 1) // nS
            while wi < target:
                kind, a = w_units[wi]
                if kind == "v8":
                    v8_unit(a)
                elif kind == "qt":
                    proj_unit(wqT_sb, qt_sb, a)
                elif kind == "kt":
                    proj_unit(wkT_sb, kt_sb, a)
                elif kind == "isl":
                    isl_unit()
                else:
                    vb_unit()
                wi += 1

        # ---- PV groups: pass A (v hi cols + den) then pass B (v lo cols);
        # rcp + A-scale hide under B's accumulation; A double-buffered ----
        pv_order = []
        for c in range(SUB):
            for I in range(nSS):
                pv_order.append((I, c))
        for (I, c) in pv_order:
                e_sb = e_strips[I]
                npv = SUB * I + c + 1
                psa = psA.tile([P, CH], F32, tag="attA")
                dpool = psW if (SUB * I + c) % 2 == 0 else psB
                dtag = "wrm" if (SUB * I + c) % 2 == 0 else "attB"
                psd = dpool.tile([P, 4], F32, tag=dtag, name="psd")
                psb = psS.tile([P, CH], F32, tag="sch", name="psb")
                row0 = I * SS + c * P
                rcp = stat.tile([P, 1], F32, tag="rcp")
                o_hi = opool.tile([P, CH], F32, tag="o")
                o_lo = opool.tile([P, CH], F32, tag="o")
                if I == 0 and c == 0:
                    # exact bf16 island: rows 0..127 attend only j-block 0
                    nc.tensor.matmul(psd[:, 0:1], eb_sb[:],
                                     vb_sb[:, DV:DV + 1],
                                     start=True, stop=True)
                    nc.vector.reciprocal(rcp[:], psd[:, 0:1])
                    nc.tensor.matmul(psa[:, 0:CH], eb_sb[:], vb_sb[:, CH:DV],
                                     start=True, stop=True)
                    nc.tensor.matmul(psb[:, 0:CH], eb_sb[:], vb_sb[:, 0:CH],
                                     start=True, stop=True)
                else:
                    odd = npv % 2
                    if odd:
                        # j-block 0 rides the zero-padded pair
                        nc.gpsimd.tensor_copy(epad[:, 0],
                                              e_sb[:, 0, c * P:(c + 1) * P])
                        stats = [(epad[:, :, :], 0)]
                        stats += [(e_sb[:, jt:jt + 2, c * P:(c + 1) * P], jt)
                                  for jt in range(1, npv - 1, 2)]
                    else:
                        stats = [(e_sb[:, jt:jt + 2, c * P:(c + 1) * P], jt)
                                 for jt in range(0, npv - 1, 2)]
                    # denominator pass first: the reciprocal runs on DVE while
                    # the PE streams the wide A/B passes
                    for ji, (st, jt) in enumerate(stats):
                        first, last = (ji == 0), (ji == len(stats) - 1)
                        nc.tensor.matmul(psd[:, 0:1], st,
                                         v_sb[:, jt:jt + 2, DV:DV + 1],
                                         start=first, stop=last,
                                         perf_mode=DR)
                    nc.vector.reciprocal(rcp[:], psd[:, 0:1])
                    for ji, (st, jt) in enumerate(stats):
                        first, last = (ji == 0), (ji == len(stats) - 1)
                        nc.tensor.matmul(psa[:, 0:CH], st,
                                         v_sb[:, jt:jt + 2, CH:DV],
                                         start=first, stop=last,
                                         perf_mode=DR)
                    for ji, (st, jt) in enumerate(stats):
                        first, last = (ji == 0), (ji == len(stats) - 1)
                        nc.tensor.matmul(psb[:, 0:CH], st,
                                         v_sb[:, jt:jt + 2, 0:CH],
                                         start=first, stop=last,
                                         perf_mode=DR)
                if (SUB * I + c) % 2 == 0:
                    nc.vector.tensor_scalar_mul(o_hi[:], psa[:, 0:CH], rcp[:])
                    nc.scalar.activation(o_lo[:], psb[:, 0:CH],
                                         ACTF.Copy, scale=rcp[:])
                else:
                    nc.scalar.activation(o_hi[:], psa[:, 0:CH],
                                         ACTF.Copy, scale=rcp[:])
                    nc.vector.tensor_scalar_mul(o_lo[:], psb[:, 0:CH], rcp[:])
                nc.sync.dma_start(out[row0:row0 + P, CH:DV], o_hi[:])
                nc.sync.dma_start(out[row0:row0 + P, 0:CH], o_lo[:])

    nc.compile()
    return nc


_NC_CACHE = {}


def _get_nc():
    if "nc" not in _NC_CACHE:
        _NC_CACHE["nc"] = _build_nc()
    return _NC_CACHE["nc"]


def kernel(x, Wq, Wk, Wv):
    x = np.asarray(x, dtype=np.float32)
    Wq = np.asarray(Wq, dtype=np.float32)
    Wk = np.asarray(Wk, dtype=np.float32)
    Wv = np.asarray(Wv, dtype=np.float32)
    assert x.shape == (B, N, D), x.shape

    nc = _get_nc()
    bf = ml_dtypes.bfloat16
    f8 = ml_dtypes.float8_e4m3
    norm = np.float32(1.0) / np.sqrt(np.float32(DK))
    wq8_h = np.ascontiguousarray(Wq * (norm * SC_WQ)).astype(f8)
    wk8_h = np.ascontiguousarray(Wk * SC_WK).astype(f8)
    wvb_h = np.ascontiguousarray(Wv.T).astype(bf)
    wqT_h = np.ascontiguousarray((Wq * norm).T).astype(bf)
    wkT_h = np.ascontiguousarray(Wk.T).astype(bf)
    in_maps = []
    for b in range(B):
        xt = np.ascontiguousarray(x[b].T)
        in_maps.append({
            "xT8": xt.astype(f8),
            "xTb": np.ascontiguousarray(xt[:, :P]).astype(bf),
            "wq8": wq8_h, "wk8": wk8_h, "wvb": wvb_h,
            "wqT": wqT_h, "wkT": wkT_h,
        })
    res = run_bass_kernel_spmd(nc, in_maps, list(range(N_CORES)))
    return np.stack([res.results[b]["out"] for b in range(B)], axis=0)
